# revision 14
# baseline (speedup 1.0000x reference)
"""CTRNN + output projection, Trainium2 Bass kernel (8-core data-parallel over batch).

Problem (hardcoded): x (2048, 512, 1) f32; W_ih (128,1); b_ih (128,); W_hh (128,128);
b_hh (128,); W_fc (1,128); b_fc (1,). alpha = 0.01/100.

Reference recurrence (per step t, h0 = 0):
    h_new = relu(x_t @ W_ih.T + b_ih + h @ W_hh.T + b_hh)
    h     = h*(1-a) + h_new*a            -> rnn_output[t] = h
    out   = relu(rnn_output @ W_fc.T + b_fc)

Device algorithm (per core, Bc = 64 batch, lam = 1-a):
  Rescaled state s_t = lam^{-t} h_t removes the decay-multiply from the critical
  path:   s_{t+1} = s_t + (a/lam) relu(W_hh s_t + lam^{-t}(W_ih x_t + b)),
  h_{t+1} = lam^{t+1} s_{t+1}.
  Per step: PE accumulates a fresh PSUM bank m = x-term (bf16 K=2 matmul of
  host-prescaled rows) + W_hh @ s (f32); DVE does r = max(m,0)*(a/lam) (fused),
  s += r, and hstage = s * lam^{t+1}; every 2 steps PE projects W_fc @ hstage
  into a small PSUM tile which ACT finishes as relu(q + b_fc) into a collector;
  staged 2 MiB DMAs stream rnn out.
"""

import numpy as np
import ml_dtypes

T, B, H, NCORES = 2048, 512, 128, 8
Bc = B // NCORES
ALPHA = 0.01 / 100.0
LAM = 1.0 - ALPHA
CHUNK = 64  # steps per hstage buffer / DMA


def _build(b_fc0: float, T: int = T):
    import concourse.bass as bass
    import concourse.mybir as mybir
    from concourse.tile import TileContext

    dt = mybir.dt
    Alu = mybir.AluOpType
    Act = mybir.ActivationFunctionType

    lam_pow = np.power(np.float64(LAM), np.arange(T + 2))

    nc = bass.Bass()
    xr_d = nc.dram_tensor("xrows", [24, (T // 4) * Bc], dt.float16, kind="ExternalInput")
    lx_d = nc.dram_tensor("lx", [24, H], dt.float16, kind="ExternalInput")
    whh_d = nc.dram_tensor("whh_t", [H, H], dt.float32, kind="ExternalInput")
    wfc_d = nc.dram_tensor("wfc_t", [H, 1], dt.float32, kind="ExternalInput")
    rnn_d = nc.dram_tensor("rnn_s", [H, T, Bc], dt.float32, kind="ExternalOutput")
    out_d = nc.dram_tensor("out_s", [1, T * Bc], dt.float32, kind="ExternalOutput")

    with TileContext(nc) as tc:
        with (
            tc.tile_pool(name="const", bufs=1) as cpool,
            tc.tile_pool(name="xp", bufs=1) as xpool,
            tc.tile_pool(name="state", bufs=1) as spool,
            tc.tile_pool(name="rp", bufs=4) as rpool,
            tc.tile_pool(name="hp", bufs=2) as hpool,
            tc.tile_pool(name="collp", bufs=2) as collpool,
            tc.tile_pool(name="mp", bufs=4, space="PSUM") as mpool,
            tc.tile_pool(name="qp", bufs=2, space="PSUM") as qpool,
        ):
            whh = cpool.tile([H, H], dt.float32)
            nc.sync.dma_start(out=whh[:], in_=whh_d[:])
            wfc = cpool.tile([H, 1], dt.float32)
            nc.sync.dma_start(out=wfc[:], in_=wfc_d[:])
            lx = cpool.tile([128, H], dt.float16)
            xr = xpool.tile([128, (T // 4) * Bc], dt.float16)
            for g in range(4):
                nc.sync.dma_start(out=lx[32 * g : 32 * g + 6, :], in_=lx_d[6 * g : 6 * g + 6, :])
                nc.sync.dma_start(out=xr[32 * g : 32 * g + 6, :], in_=xr_d[6 * g : 6 * g + 6, :])

            s = spool.tile([H, Bc], dt.float32)
            nc.vector.memset(s[:], 0.0)
            bfc_t = cpool.tile([1, 1], dt.float32)
            nc.vector.memset(bfc_t[:], float(b_fc0))

            relu_scale = float(ALPHA / LAM)
            hst = None
            coll = None
            for t in range(T):
                c = t % CHUNK
                if c == 0:
                    hst = hpool.tile([H, CHUNK * Bc], dt.float32, tag="hst", name="hst")
                g, i = t % 4, t // 4
                m = mpool.tile([H, Bc], dt.float32, tag="m", name="m")
                nc.tensor.matmul(
                    m[:],
                    lx[32 * g : 32 * g + 6, :],
                    xr[32 * g : 32 * g + 6, i * Bc : (i + 1) * Bc],
                    start=True,
                    stop=False,
                    tile_position=(32 * g, 0),
                )
                nc.tensor.matmul(m[:], whh[:], s[:], start=False, stop=True)
                r = rpool.tile([H, Bc], dt.float32, tag="r", name="r")
                nc.vector.tensor_scalar(r[:], m[:], 0.0, relu_scale, Alu.max, Alu.mult)
                nc.vector.tensor_tensor(s[:], s[:], r[:], Alu.add)
                nc.vector.tensor_scalar(
                    hst[:, c * Bc : (c + 1) * Bc], s[:], float(lam_pow[t + 1]), None, Alu.mult
                )
                if t % 2 == 1:
                    c2 = t // 2
                    if c2 % 32 == 0:
                        coll = collpool.tile([1, 32 * 2 * Bc], dt.float32, tag="coll", name="coll")
                    q = qpool.tile([1, 2 * Bc], dt.float32, tag="q", name="q")
                    nc.tensor.matmul(
                        q[:], wfc[:], hst[:, (c - 1) * Bc : (c + 1) * Bc], start=True, stop=True
                    )
                    nc.scalar.activation(
                        coll[:, (c2 % 32) * 2 * Bc : (c2 % 32 + 1) * 2 * Bc],
                        q[:],
                        Act.Relu,
                        bias=bfc_t[:],
                        scale=1.0,
                    )
                if c == CHUNK - 1:
                    t0 = t - (CHUNK - 1)
                    nc.sync.dma_start(
                        out=rnn_d[:, t0 : t + 1, :],
                        in_=hst[:].rearrange("p (c b) -> p c b", b=Bc),
                    )
                    nc.sync.dma_start(
                        out=out_d[:, t0 * Bc : (t + 1) * Bc], in_=coll[:]
                    )
    import bass_rust
    # Walrus allows at most 1 sync wait per instruction (2 on EventSemaphore);
    # split excess waits the way Bacc.compile() does.
    bass_rust.generate_event_semaphores(nc)
    return nc


def _host_inputs(x, W_ih, b_ih, W_hh, b_hh, W_fc, b_fc, T: int = T):
    """Build per-core input maps. x: (T, B, 1) f32."""
    f16 = np.float16

    def hl(a):
        hi = a.astype(f16)
        lo = (a - hi.astype(np.float32)).astype(f16)
        return hi, lo

    lam_inv = np.power(np.float64(LAM), -np.arange(T))  # lam^-t
    bc = (b_ih + b_hh).astype(np.float32)
    w_ih = W_ih[:, 0].astype(np.float32)
    w_hi, w_lo = hl(w_ih)
    bc_hi, bc_lo = hl(bc)

    # lhsT rows per strip: [W_hi, W_hi, W_lo, bc_hi, bc_hi, bc_lo] pair with
    # rhs rows [xhat_hi, xhat_lo, xhat_hi, lhat_hi, lhat_lo, lhat_hi].
    lx = np.zeros((24, H), f16)
    for g in range(4):
        lx[6 * g + 0] = w_hi
        lx[6 * g + 1] = w_hi
        lx[6 * g + 2] = w_lo
        lx[6 * g + 3] = bc_hi
        lx[6 * g + 4] = bc_hi
        lx[6 * g + 5] = bc_lo

    whh_t = np.ascontiguousarray(W_hh.T.astype(np.float32))
    wfc_t = np.ascontiguousarray(W_fc[0].astype(np.float32).reshape(H, 1))

    xs = (lam_inv[:, None] * x[:, :, 0].astype(np.float64)).astype(np.float32)  # (T,B)
    ls = lam_inv.astype(np.float32)  # (T,)
    ls_hi, ls_lo = hl(ls)

    in_maps = []
    for core in range(NCORES):
        xc = xs[:, core * Bc : (core + 1) * Bc]  # (T, Bc)
        xc_hi, xc_lo = hl(xc)
        xr = np.zeros((24, (T // 4) * Bc), f16)
        for g in range(4):
            tsel = np.arange(g, T, 4)  # t = 4i + g
            xr[6 * g + 0] = xc_hi[tsel].reshape(-1)  # [i, b] flat
            xr[6 * g + 1] = xc_lo[tsel].reshape(-1)
            xr[6 * g + 2] = xc_hi[tsel].reshape(-1)
            xr[6 * g + 3] = np.repeat(ls_hi[tsel], Bc)
            xr[6 * g + 4] = np.repeat(ls_lo[tsel], Bc)
            xr[6 * g + 5] = np.repeat(ls_hi[tsel], Bc)
        in_maps.append(
            {
                "xrows": xr,
                "lx": lx,
                "whh_t": whh_t,
                "wfc_t": wfc_t,
            }
        )
    return in_maps


_CACHE = {}
LAST_RESULT = None


def kernel(x, W_ih, b_ih, W_hh, b_hh, W_fc, b_fc):
    from concourse.bass_utils import run_bass_kernel_spmd

    x = np.asarray(x, np.float32)
    b_fc0 = float(np.asarray(b_fc, np.float32)[0])

    key = ("nc", b_fc0)
    if key not in _CACHE:
        _CACHE[key] = _build(b_fc0)
    nc = _CACHE[key]

    in_maps = _host_inputs(
        x,
        np.asarray(W_ih, np.float32),
        np.asarray(b_ih, np.float32),
        np.asarray(W_hh, np.float32),
        np.asarray(b_hh, np.float32),
        np.asarray(W_fc, np.float32),
        np.asarray(b_fc, np.float32),
    )
    res = run_bass_kernel_spmd(nc, in_maps, list(range(NCORES)))
    global LAST_RESULT
    LAST_RESULT = res
    results = res.results

    lam_pow = None  # rnn already lam-scaled on device
    rnn_parts, out_parts = [], []
    for core in range(NCORES):
        rnn_s = np.asarray(results[core]["rnn_s"])  # (H, T, Bc)
        out_s = np.asarray(results[core]["out_s"]).reshape(T, Bc, 1)
        rnn_parts.append(np.transpose(rnn_s, (1, 2, 0)))  # (T, Bc, H)
        out_parts.append(out_s)
    rnn = np.concatenate(rnn_parts, axis=1).astype(np.float32)
    out = np.concatenate(out_parts, axis=1).astype(np.float32)
    return out, rnn
